# revision 7
# baseline (speedup 1.0000x reference)
"""GQA attention kernel for Trainium2, 8 NeuronCores.

Sharding: 2 batches x 4 kv-head groups = 8 cores. Each core computes, for its
batch b and kv group g (4 query heads, 1 kv head):
    Q = x_b @ Wq[:, g]     (512 cols)      K = x_b @ Wk[:, g] (128 cols)
    V = x_b @ Wv[:, g]     (128 cols)
    A_h = softmax_causal(Q_h K^T / sqrt(128)) V        (h = 4 heads)
    Y_partial = concat_h(A_h) @ Wo[rows g]             [2048, 2048]

Host<->device traffic is the wall-clock bottleneck on this runner (axon
tunnel, ~tens of MB/s), so every payload byte is shipped exactly once:
  - x arrives 4-way sharded per batch group ([512, 2048] bf16 per core) and
    is AllGathered on device within groups {0-3} / {4-7}.
  - Weight shards are identical for the two batch groups, so each core gets
    half and pair-AllGathers with its twin ({g, g+4}).
  - The 4 per-group Y partials are ReduceScattered on device; each core
    returns only its 512-row share, in bf16. Host concatenates and adds bo.
  - Masks/identity/ones are inlined into the NEFF as Const data.

Device layout choices (all matmul operands natural, no transposes in hot loop):
  xT [d, t] fed sharded from host; QT/KT computed transposed ([e, t]); V
  non-transposed via PE transpose of VT; scores computed transposed ST [s, t]
  so that AV (lhsT=V[s,e], rhs=expST[s,t]) and O-proj (lhsT=attnT[c,t],
  rhs=Wo[c,f]) need no on-device transposition. Softmax denominators via
  ones-vector matmuls; normalization deferred to attnT evacuation using a
  PE-broadcast of 1/Z. Causal masking: only lower-triangular 128x512 score
  blocks are computed; diagonal blocks masked multiplicatively post-exp.
Compute dtype bf16 (inputs cast on host), accumulation f32.
"""

import os
import sys

sys.path.insert(0, "/opt/trn_rl_repo")

import numpy as np
import ml_dtypes

import jax

# Every run_bass_kernel_spmd call builds a fresh jax.jit, so without a
# persistent compilation cache each call pays ~0.2-0.6s of XLA re-compile.
# With it, warm calls hit disk (~10ms).
jax.config.update("jax_compilation_cache_dir", "/tmp/jaxcache_gqa")
jax.config.update("jax_persistent_cache_min_compile_time_secs", 0.0)
jax.config.update("jax_persistent_cache_min_entry_size_bytes", 0)

import concourse.bass as bass
from concourse import bacc
import concourse.tile as tile
from concourse import mybir
from concourse.bass_utils import run_bass_kernel_spmd

BF = mybir.dt.bfloat16
F32 = mybir.dt.float32

D = 2048        # d_model
T = 2048        # seq len
B = 2
NUM_HEADS = 16
NUM_KV = 4
DH = 128        # head dim
HPG = NUM_HEADS // NUM_KV   # 4 query heads per core
EG = HPG * DH               # 512 q-channels per core
TS = 512                    # t-slice width (phase A psum tiles, phase B rhs)
NT = T // TS                # 4
NJ = D // 128               # 16 contraction chunks / s-tiles
SCALE = 1.0 / float(np.sqrt(DH))

GRP_BATCH = [[0, 1, 2, 3], [4, 5, 6, 7]]   # same batch, 4 kv groups
GRP_PAIR = [[0, 4], [1, 5], [2, 6], [3, 7]]  # same kv group, 2 batches

_NC_CACHE = {}


def _cpack():
    bf = ml_dtypes.bfloat16
    ident = np.eye(128, dtype=np.float32)
    tc = np.arange(512)[None, :]
    s = np.arange(128)[:, None]
    def mk(o0, o1):
        return np.concatenate(
            [(tc >= o0 * 128 + s), (tc >= o1 * 128 + s)], axis=1
        ).astype(np.float32)
    ones = np.ones((128, 129), np.float32)
    return np.concatenate([ident, mk(0, 1), mk(2, 3), ones], axis=1).astype(bf)


def build_nc():
    if "nc" in _NC_CACHE:
        return _NC_CACHE["nc"]
    nc = bass.Bass(num_devices=8)
    # One packed input per core ([2305, 1024] bf16): fewer, larger tunnel
    # transfers. Rows 0:1024 xT quarter | 1024:1536 Wq half | 1536:1792
    # [Wk|Wv] half | 1792:2304 Wo half | 2304 biases (bf16, first 768 cols).
    pk = nc.dram_tensor("pk", [2305, 1024], BF, kind="ExternalInput").ap()
    y = nc.dram_tensor("y", [TS, D], BF, kind="ExternalOutput").ap()
    cpack_d = nc.inline_tensor(_cpack(), name="cpack")

    with tile.TileContext(nc) as tc:
        with (
            tc.tile_pool(name="dio", bufs=1, space="DRAM") as dio,
            tc.tile_pool(name="consts", bufs=1) as consts,
            tc.tile_pool(name="persist", bufs=1) as persist,
            tc.tile_pool(name="wpool", bufs=1) as wpool,
            tc.tile_pool(name="xpool", bufs=64) as xpool,
            tc.tile_pool(name="expp", bufs=3) as expp,
            tc.tile_pool(name="attp", bufs=8) as attp,
            tc.tile_pool(name="ypool", bufs=4) as ypool,
            tc.tile_pool(name="small", bufs=8) as small,
        ):
            # ---- gather sharded inputs on device ---------------------------
            # Bounce tiles mirror flat pack rows (collectives cannot read
            # kernel I/O); AllGather concatenates flat buffers, so the AG
            # outputs are declared with their natural 2D shapes.
            xb = dio.tile([1024, 1024], BF)
            xF = dio.tile([D, T], BF)          # full xT for this batch
            wqb = dio.tile([512, 1024], BF)
            WqF = dio.tile([D, EG], BF)
            wkvb = dio.tile([256, 1024], BF)
            WkvF = dio.tile([D, 2 * DH], BF)
            wob = dio.tile([512, 1024], BF)
            WoF = dio.tile([EG, D], BF)
            yp = dio.tile([T, D], BF)          # this core's Y partial
            yrs = dio.tile([TS, D], BF)        # reduce-scattered share

            nc.gpsimd.dma_start(xb[:], pk[0:1024, :])
            nc.gpsimd.collective_compute(
                "AllGather", mybir.AluOpType.bypass, replica_groups=GRP_BATCH,
                ins=[xb.opt()], outs=[xF.opt()],
            )
            nc.gpsimd.dma_start(wqb[:], pk[1024:1536, :])
            nc.gpsimd.collective_compute(
                "AllGather", mybir.AluOpType.bypass, replica_groups=GRP_PAIR,
                ins=[wqb.opt()], outs=[WqF.opt()],
            )
            nc.gpsimd.dma_start(wkvb[:], pk[1536:1792, :])
            nc.gpsimd.collective_compute(
                "AllGather", mybir.AluOpType.bypass, replica_groups=GRP_PAIR,
                ins=[wkvb.opt()], outs=[WkvF.opt()],
            )
            nc.gpsimd.dma_start(wob[:], pk[1792:2304, :])
            nc.gpsimd.collective_compute(
                "AllGather", mybir.AluOpType.bypass, replica_groups=GRP_PAIR,
                ins=[wob.opt()], outs=[WoF.opt()],
            )

            # ---- constants -------------------------------------------------
            # cpack: identity(128) | maskA(1024) | maskB(1024) | ones(1) | ones(128)
            cpack = consts.tile([128, 2305], BF)
            nc.sync.dma_start(out=cpack, in_=cpack_d[:])
            bbf = consts.tile([128, 6], BF)
            nc.sync.dma_start(
                out=bbf,
                in_=pk[2304:2305, 0:768].rearrange("a (p c) -> (a p) c", p=128),
            )
            bpack = consts.tile([128, 6], F32)
            nc.vector.tensor_copy(out=bpack, in_=bbf)
            identity = cpack[:, 0:128]
            maskA = cpack[:, 128:1152]     # diag-block masks, offsets 0,1
            maskB = cpack[:, 1152:2176]    # offsets 2,3
            ones_s = cpack[:, 2176:2177]   # lhsT for column sums
            ones_r = cpack[0:1, 2177:2305]  # lhsT for partition bcast
            bq_sb = bpack[:, 0:HPG]
            bk_sb = bpack[:, HPG:HPG + 1]
            bv_sb = bpack[:, HPG + 1:HPG + 2]
            # Pre-touch on DVE: later DVE consumers then carry only one wait
            # (walrus allows a single sync wait on DVE tensor_scalar ops).
            pt = consts.tile([128, 16], BF)
            nc.vector.tensor_copy(out=pt, in_=cpack[:, 0:16])
            ptf = consts.tile([128, 6], F32)
            nc.vector.tensor_copy(out=ptf, in_=bpack)

            # ---- persistent activations -----------------------------------
            QT = [persist.tile([128, T], BF, tag=f"QT{h}", name=f"QT{h}") for h in range(HPG)]
            KT = persist.tile([128, T], BF, tag="KT")
            V = persist.tile([128, NJ, DH], BF, tag="V")       # [s%128, j, e]
            Wq_sb = wpool.tile([128, NJ, EG], BF, tag="Wq")
            Wk_sb = wpool.tile([128, NJ, DH], BF, tag="Wk")
            Wv_sb = wpool.tile([128, NJ, DH], BF, tag="Wv")
            Wo_sb = wpool.tile([128, HPG, D], BF, tag="Wo")    # [c%128, h, f]
            for h in range(HPG):
                nc.sync.dma_start(out=Wo_sb[:, h, :], in_=WoF[h * 128:(h + 1) * 128, :])

            # ---- phase A: projections QT/KT/V ------------------------------
            with (
                tc.tile_pool(name="psA", bufs=1, space="PSUM") as psA,
                tc.tile_pool(name="psAv", bufs=2, space="PSUM") as psAv,
            ):
                warm = psAv.tile([128, 128], BF, tag="v_ps")
                nc.tensor.transpose(warm, identity, identity)
                for Tt in range(NT):
                    tsl = slice(Tt * TS, (Tt + 1) * TS)
                    xa = []
                    for j in range(NJ):
                        xt = xpool.tile([128, TS], BF, tag="xa")
                        nc.sync.dma_start(out=xt, in_=xF[j * 128:(j + 1) * 128, tsl])
                        xa.append(xt)
                        if Tt == 0:
                            nc.sync.dma_start(out=Wq_sb[:, j, :], in_=WqF[j * 128:(j + 1) * 128, :])
                            nc.sync.dma_start(out=Wk_sb[:, j, :], in_=WkvF[j * 128:(j + 1) * 128, 0:DH])
                            nc.sync.dma_start(out=Wv_sb[:, j, :], in_=WkvF[j * 128:(j + 1) * 128, DH:2 * DH])
                    # one output tile at a time so evacuation overlaps compute
                    for h in range(HPG):
                        qt_ps = psA.tile([128, TS], F32, tag=f"qt{h}")
                        for j in range(NJ):
                            nc.tensor.matmul(
                                qt_ps, Wq_sb[:, j, h * 128:(h + 1) * 128], xa[j],
                                start=(j == 0), stop=(j == NJ - 1),
                            )
                        nc.vector.tensor_scalar_add(
                            out=QT[h][:, tsl], in0=qt_ps,
                            scalar1=bq_sb[:, h:h + 1],
                        )
                    kt_ps = psA.tile([128, TS], F32, tag="kt")
                    for j in range(NJ):
                        nc.tensor.matmul(kt_ps, Wk_sb[:, j, :], xa[j],
                                         start=(j == 0), stop=(j == NJ - 1))
                    nc.vector.tensor_scalar_add(
                        out=KT[:, tsl], in0=kt_ps, scalar1=bk_sb,
                    )
                    vt_ps = psA.tile([128, TS], F32, tag="vt")
                    for j in range(NJ):
                        nc.tensor.matmul(vt_ps, Wv_sb[:, j, :], xa[j],
                                         start=(j == 0), stop=(j == NJ - 1))
                    vt_sb = small.tile([128, TS], BF, tag="vt_sb")
                    nc.vector.tensor_scalar_add(
                        out=vt_sb, in0=vt_ps, scalar1=bv_sb,
                    )
                    # VT [e, t] -> V [t, e] per 128-block via PE transpose
                    for k in range(TS // 128):
                        v_ps = psAv.tile([128, 128], BF, tag="v_ps")
                        nc.tensor.transpose(v_ps, vt_sb[:, k * 128:(k + 1) * 128], identity)
                        nc.vector.tensor_copy(out=V[:, Tt * 4 + k, :], in_=v_ps)

            # ---- phase B/C: attention + output projection ------------------
            with (
                tc.tile_pool(name="psst", bufs=2, space="PSUM") as psst,
                tc.tile_pool(name="psat", bufs=1, space="PSUM") as psat,
                tc.tile_pool(name="psz", bufs=1, space="PSUM") as psz,
                tc.tile_pool(name="psy", bufs=2, space="PSUM") as psy,
            ):
                for Tt in range(NT):
                    tsl = slice(Tt * TS, (Tt + 1) * TS)
                    att_sb = []
                    for h in range(HPG):
                        njj = 4 * Tt + 4          # s-tiles 0 .. 4*Tt+3
                        ngr = njj // 2
                        at_ps = psat.tile([128, TS], F32, tag="at")
                        z_ps = psz.tile([1, TS], F32, tag="z")
                        for g in range(ngr):
                            j0 = 2 * g
                            st = psst.tile([128, 1024], F32, tag="st")
                            for half in range(2):
                                j = j0 + half
                                nc.tensor.matmul(
                                    st[:, half * 512:(half + 1) * 512],
                                    KT[:, j * 128:(j + 1) * 128],
                                    QT[h][:, tsl],
                                    start=True, stop=True,
                                )
                            ex = expp.tile([128, 1024], BF, tag="ex")
                            nc.scalar.activation(
                                out=ex, in_=st,
                                func=mybir.ActivationFunctionType.Exp,
                                scale=SCALE,
                            )
                            if g == ngr - 2:
                                nc.vector.tensor_mul(ex, ex, maskA)
                            elif g == ngr - 1:
                                nc.vector.tensor_mul(ex, ex, maskB)
                            for half in range(2):
                                j = j0 + half
                                exh = ex[:, half * 512:(half + 1) * 512]
                                nc.tensor.matmul(
                                    z_ps, ones_s, exh,
                                    start=(j == 0), stop=(j == njj - 1),
                                )
                                nc.tensor.matmul(
                                    at_ps, V[:, j, :], exh,
                                    start=(j == 0), stop=(j == njj - 1),
                                )
                        zr = small.tile([1, TS], F32, tag="zr")
                        nc.vector.reciprocal(out=zr, in_=z_ps)
                        zrb = small.tile([1, TS], BF, tag="zrb")
                        nc.vector.tensor_copy(out=zrb, in_=zr)
                        zb_ps = psz.tile([128, TS], F32, tag="z")
                        nc.tensor.matmul(zb_ps, ones_r, zrb,
                                         start=True, stop=True)
                        zb_sb = small.tile([128, TS], BF, tag="zb_sb")
                        nc.vector.tensor_copy(out=zb_sb, in_=zb_ps)
                        at_sb = attp.tile([128, TS], BF, tag="at_sb")
                        nc.vector.tensor_mul(at_sb, at_ps, zb_sb)
                        att_sb.append(at_sb)
                    # output projection for these 512 rows
                    for fs in range(4):
                        fsl = slice(fs * 512, (fs + 1) * 512)
                        for tt in range(4):
                            y_ps = psy.tile([128, 512], F32, tag="y")
                            for h in range(HPG):
                                nc.tensor.matmul(
                                    y_ps,
                                    att_sb[h][:, tt * 128:(tt + 1) * 128],
                                    Wo_sb[:, h, fsl],
                                    start=(h == 0), stop=(h == HPG - 1),
                                )
                            y_sb = ypool.tile([128, 512], BF, tag="y_sb")
                            nc.vector.tensor_copy(out=y_sb, in_=y_ps)
                            nc.sync.dma_start(
                                out=yp[Tt * TS + tt * 128: Tt * TS + (tt + 1) * 128, fsl],
                                in_=y_sb,
                            )

            # ---- reduce partials across the 4 kv groups, return our share --
            nc.gpsimd.collective_compute(
                "ReduceScatter", mybir.AluOpType.add, replica_groups=GRP_BATCH,
                ins=[yp.opt()], outs=[yrs.opt()],
            )
            nc.gpsimd.dma_start(y, yrs[:])
    from concourse.bacc import _bass_rust
    _bass_rust.move_matmul_waits_to_ldweights(nc.m)
    _bass_rust.generate_event_semaphores(nc)
    _NC_CACHE["nc"] = nc
    return nc


def make_in_maps(x, Wq, bq, Wk, bk, Wv, bv, Wo, bo):
    bf = ml_dtypes.bfloat16
    x = np.asarray(x)
    xT = [np.ascontiguousarray(x[b].T, dtype=bf) for b in range(B)]
    Wq_bf = np.asarray(Wq).astype(bf)
    Wk_bf = np.asarray(Wk).astype(bf)
    Wv_bf = np.asarray(Wv).astype(bf)
    Wo_bf = np.asarray(Wo).astype(bf)
    bq = np.asarray(bq, dtype=np.float32)
    bk = np.asarray(bk, dtype=np.float32)
    bv = np.asarray(bv, dtype=np.float32)
    in_maps = []
    for c in range(8):
        b, g = divmod(c, NUM_KV)
        rq = slice(b * (D // 2), (b + 1) * (D // 2))
        pk = np.zeros((2305, 1024), dtype=bf)
        pk[0:1024] = xT[b][g * TS:(g + 1) * TS].reshape(1024, 1024)
        pk[1024:1536] = Wq_bf[rq, g * EG:(g + 1) * EG].reshape(512, 1024)
        pk[1536:1792] = np.concatenate(
            [Wk_bf[rq, g * DH:(g + 1) * DH],
             Wv_bf[rq, g * DH:(g + 1) * DH]], axis=1).reshape(256, 1024)
        pk[1792:2304] = Wo_bf[
            g * EG + b * (EG // 2): g * EG + (b + 1) * (EG // 2), :
        ].reshape(512, 1024)
        bp = np.concatenate(
            [bq[g * EG:(g + 1) * EG].reshape(4, DH).T,
             bk[g * DH:(g + 1) * DH].reshape(DH, 1),
             bv[g * DH:(g + 1) * DH].reshape(DH, 1)], axis=1)
        pk[2304, 0:768] = bp.astype(bf).reshape(768)
        in_maps.append({"pk": pk})
    return in_maps


def gather(results, bo):
    bo = np.asarray(bo, dtype=np.float32)
    out = np.empty((B, T, D), dtype=np.float32)
    for b in range(B):
        for g in range(NUM_KV):
            out[b, g * TS:(g + 1) * TS] = results[b * NUM_KV + g]["y"]
        out[b] += bo[None, :]
    return out


def kernel(x, Wq, bq, Wk, bk, Wv, bv, Wo, bo):
    nc = build_nc()
    in_maps = make_in_maps(x, Wq, bq, Wk, bk, Wv, bv, Wo, bo)
    last = None
    for attempt in range(3):
        try:
            res = run_bass_kernel_spmd(nc, in_maps, list(range(8)))
            return gather(res.results, bo)
        except Exception as e:  # transient NRT_EXEC_UNIT_UNRECOVERABLE
            last = e
            import time as _t
            _t.sleep(10)
    raise last


# revision 11
# speedup vs baseline: 1.0095x; 1.0095x over previous
"""GQA attention kernel for Trainium2, 8 NeuronCores.

Sharding: 2 batches x 4 kv-head groups = 8 cores. Each core computes, for its
batch b and kv group g (4 query heads, 1 kv head):
    Q = x_b @ Wq[:, g]     (512 cols)      K = x_b @ Wk[:, g] (128 cols)
    V = x_b @ Wv[:, g]     (128 cols)
    A_h = softmax_causal(Q_h K^T / sqrt(128)) V        (h = 4 heads)
    Y_partial = concat_h(A_h) @ Wo[rows g]             [2048, 2048]

Host<->device traffic is the wall-clock bottleneck on this runner (axon
tunnel, ~tens of MB/s), so every payload byte is shipped exactly once:
  - x arrives 4-way sharded per batch group ([512, 2048] bf16 per core) and
    is AllGathered on device within groups {0-3} / {4-7}.
  - Weight shards are identical for the two batch groups, so each core gets
    half and pair-AllGathers with its twin ({g, g+4}).
  - The 4 per-group Y partials are ReduceScattered on device; each core
    returns only its 512-row share, in bf16. Host concatenates and adds bo.
  - Masks/identity/ones are inlined into the NEFF as Const data.

Device layout choices (all matmul operands natural, no transposes in hot loop):
  xT [d, t] fed sharded from host; QT/KT computed transposed ([e, t]); V
  non-transposed via PE transpose of VT; scores computed transposed ST [s, t]
  so that AV (lhsT=V[s,e], rhs=expST[s,t]) and O-proj (lhsT=attnT[c,t],
  rhs=Wo[c,f]) need no on-device transposition. Softmax denominators via
  ones-vector matmuls; normalization deferred to attnT evacuation using a
  PE-broadcast of 1/Z. Causal masking: only lower-triangular 128x512 score
  blocks are computed; diagonal blocks masked multiplicatively post-exp.
Compute dtype bf16 (inputs cast on host), accumulation f32.
"""

import os
import sys

sys.path.insert(0, "/opt/trn_rl_repo")

import numpy as np
import ml_dtypes

import jax

# Every run_bass_kernel_spmd call builds a fresh jax.jit, so without a
# persistent compilation cache each call pays ~0.2-0.6s of XLA re-compile.
# With it, warm calls hit disk (~10ms).
jax.config.update("jax_compilation_cache_dir", "/tmp/jaxcache_gqa")
jax.config.update("jax_persistent_cache_min_compile_time_secs", 0.0)
jax.config.update("jax_persistent_cache_min_entry_size_bytes", 0)

import concourse.bass as bass
from concourse import bacc
import concourse.tile as tile
from concourse import mybir
from concourse.bass_utils import run_bass_kernel_spmd

BF = mybir.dt.bfloat16
F32 = mybir.dt.float32

D = 2048        # d_model
T = 2048        # seq len
B = 2
NUM_HEADS = 16
NUM_KV = 4
DH = 128        # head dim
HPG = NUM_HEADS // NUM_KV   # 4 query heads per core
EG = HPG * DH               # 512 q-channels per core
TS = 512                    # t-slice width (phase A psum tiles, phase B rhs)
NT = T // TS                # 4
NJ = D // 128               # 16 contraction chunks / s-tiles
SCALE = 1.0 / float(np.sqrt(DH))

GRP_BATCH = [[0, 1, 2, 3], [4, 5, 6, 7]]   # same batch, 4 kv groups
GRP_PAIR = [[0, 4], [1, 5], [2, 6], [3, 7]]  # same kv group, 2 batches

_NC_CACHE = {}


def _cpack():
    bf = ml_dtypes.bfloat16
    ident = np.eye(128, dtype=np.float32)
    tc = np.arange(512)[None, :]
    s = np.arange(128)[:, None]
    def mk(o0, o1):
        return np.concatenate(
            [(tc >= o0 * 128 + s), (tc >= o1 * 128 + s)], axis=1
        ).astype(np.float32)
    ones = np.ones((128, 129), np.float32)
    return np.concatenate([ident, mk(0, 1), mk(2, 3), ones], axis=1).astype(bf)


def build_nc():
    if "nc" in _NC_CACHE:
        return _NC_CACHE["nc"]
    nc = bass.Bass(num_devices=8)
    xs = nc.dram_tensor("xs", [TS, T], BF, kind="ExternalInput").ap()
    Wqh = nc.dram_tensor("Wqh", [D // 2, EG], BF, kind="ExternalInput").ap()
    Wkvh = nc.dram_tensor("Wkvh", [D // 2, 2 * DH], BF, kind="ExternalInput").ap()
    Woh = nc.dram_tensor("Woh", [EG // 2, D], BF, kind="ExternalInput").ap()
    bpack_d = nc.dram_tensor("bpack", [128, 6], F32, kind="ExternalInput").ap()
    y = nc.dram_tensor("y", [TS, D], BF, kind="ExternalOutput").ap()
    cpack_d = nc.inline_tensor(_cpack(), name="cpack")

    with tile.TileContext(nc) as tc:
        with (
            tc.tile_pool(name="dio", bufs=1, space="DRAM") as dio,
            tc.tile_pool(name="consts", bufs=1) as consts,
            tc.tile_pool(name="persist", bufs=1) as persist,
            tc.tile_pool(name="wpool", bufs=1) as wpool,
            tc.tile_pool(name="xpool", bufs=64) as xpool,
            tc.tile_pool(name="expp", bufs=3) as expp,
            tc.tile_pool(name="attp", bufs=8) as attp,
            tc.tile_pool(name="ypool", bufs=4) as ypool,
            tc.tile_pool(name="small", bufs=8) as small,
        ):
            # ---- gather sharded inputs on device ---------------------------
            xb = dio.tile([TS, T], BF)
            xF = dio.tile([D, T], BF)          # full xT for this batch
            wqb = dio.tile([D // 2, EG], BF)
            WqF = dio.tile([D, EG], BF)
            wkvb = dio.tile([D // 2, 2 * DH], BF)
            WkvF = dio.tile([D, 2 * DH], BF)
            wob = dio.tile([EG // 2, D], BF)
            WoF = dio.tile([EG, D], BF)
            yp = dio.tile([T, D], BF)          # this core's Y partial
            yrs = dio.tile([TS, D], BF)        # reduce-scattered share

            nc.gpsimd.dma_start(xb[:], xs)
            nc.gpsimd.collective_compute(
                "AllGather", mybir.AluOpType.bypass, replica_groups=GRP_BATCH,
                ins=[xb.opt()], outs=[xF.opt()],
            )
            nc.gpsimd.dma_start(wqb[:], Wqh)
            nc.gpsimd.collective_compute(
                "AllGather", mybir.AluOpType.bypass, replica_groups=GRP_PAIR,
                ins=[wqb.opt()], outs=[WqF.opt()],
            )
            nc.gpsimd.dma_start(wkvb[:], Wkvh)
            nc.gpsimd.collective_compute(
                "AllGather", mybir.AluOpType.bypass, replica_groups=GRP_PAIR,
                ins=[wkvb.opt()], outs=[WkvF.opt()],
            )
            nc.gpsimd.dma_start(wob[:], Woh)
            nc.gpsimd.collective_compute(
                "AllGather", mybir.AluOpType.bypass, replica_groups=GRP_PAIR,
                ins=[wob.opt()], outs=[WoF.opt()],
            )

            # ---- constants -------------------------------------------------
            # cpack: identity(128) | maskA(1024) | maskB(1024) | ones(1) | ones(128)
            cpack = consts.tile([128, 2305], BF)
            nc.sync.dma_start(out=cpack, in_=cpack_d[:])
            bpack = consts.tile([128, 6], F32)
            nc.sync.dma_start(out=bpack, in_=bpack_d)
            identity = cpack[:, 0:128]
            maskA = cpack[:, 128:1152]     # diag-block masks, offsets 0,1
            maskB = cpack[:, 1152:2176]    # offsets 2,3
            ones_s = cpack[:, 2176:2177]   # lhsT for column sums
            ones_r = cpack[0:1, 2177:2305]  # lhsT for partition bcast
            bq_sb = bpack[:, 0:HPG]
            bk_sb = bpack[:, HPG:HPG + 1]
            bv_sb = bpack[:, HPG + 1:HPG + 2]
            # Pre-touch on DVE: later DVE consumers then carry only one wait
            # (walrus allows a single sync wait on DVE tensor_scalar ops).
            pt = consts.tile([128, 16], BF)
            nc.vector.tensor_copy(out=pt, in_=cpack[:, 0:16])
            ptf = consts.tile([128, 6], F32)
            nc.vector.tensor_copy(out=ptf, in_=bpack)

            # ---- persistent activations -----------------------------------
            QT = [persist.tile([128, T], BF, tag=f"QT{h}", name=f"QT{h}") for h in range(HPG)]
            KT = persist.tile([128, T], BF, tag="KT")
            V = persist.tile([128, NJ, DH], BF, tag="V")       # [s%128, j, e]
            Wq_sb = wpool.tile([128, NJ, EG], BF, tag="Wq")
            Wk_sb = wpool.tile([128, NJ, DH], BF, tag="Wk")
            Wv_sb = wpool.tile([128, NJ, DH], BF, tag="Wv")
            Wo_sb = wpool.tile([128, HPG, D], BF, tag="Wo")    # [c%128, h, f]
            for h in range(HPG):
                nc.sync.dma_start(out=Wo_sb[:, h, :], in_=WoF[h * 128:(h + 1) * 128, :])

            # ---- phase A: projections QT/KT/V ------------------------------
            with (
                tc.tile_pool(name="psA", bufs=1, space="PSUM") as psA,
                tc.tile_pool(name="psAv", bufs=2, space="PSUM") as psAv,
            ):
                warm = psAv.tile([128, 128], BF, tag="v_ps")
                nc.tensor.transpose(warm, identity, identity)
                for Tt in range(NT):
                    tsl = slice(Tt * TS, (Tt + 1) * TS)
                    xa = []
                    for j in range(NJ):
                        xt = xpool.tile([128, TS], BF, tag="xa")
                        nc.sync.dma_start(out=xt, in_=xF[j * 128:(j + 1) * 128, tsl])
                        xa.append(xt)
                        if Tt == 0:
                            nc.sync.dma_start(out=Wq_sb[:, j, :], in_=WqF[j * 128:(j + 1) * 128, :])
                            nc.sync.dma_start(out=Wk_sb[:, j, :], in_=WkvF[j * 128:(j + 1) * 128, 0:DH])
                            nc.sync.dma_start(out=Wv_sb[:, j, :], in_=WkvF[j * 128:(j + 1) * 128, DH:2 * DH])
                    # one output tile at a time so evacuation overlaps compute
                    for h in range(HPG):
                        qt_ps = psA.tile([128, TS], F32, tag=f"qt{h}")
                        for j in range(NJ):
                            nc.tensor.matmul(
                                qt_ps, Wq_sb[:, j, h * 128:(h + 1) * 128], xa[j],
                                start=(j == 0), stop=(j == NJ - 1),
                            )
                        nc.vector.tensor_scalar_add(
                            out=QT[h][:, tsl], in0=qt_ps,
                            scalar1=bq_sb[:, h:h + 1],
                        )
                    kt_ps = psA.tile([128, TS], F32, tag="kt")
                    for j in range(NJ):
                        nc.tensor.matmul(kt_ps, Wk_sb[:, j, :], xa[j],
                                         start=(j == 0), stop=(j == NJ - 1))
                    nc.vector.tensor_scalar_add(
                        out=KT[:, tsl], in0=kt_ps, scalar1=bk_sb,
                    )
                    vt_ps = psA.tile([128, TS], F32, tag="vt")
                    for j in range(NJ):
                        nc.tensor.matmul(vt_ps, Wv_sb[:, j, :], xa[j],
                                         start=(j == 0), stop=(j == NJ - 1))
                    vt_sb = small.tile([128, TS], BF, tag="vt_sb")
                    nc.vector.tensor_scalar_add(
                        out=vt_sb, in0=vt_ps, scalar1=bv_sb,
                    )
                    # VT [e, t] -> V [t, e] per 128-block via PE transpose
                    for k in range(TS // 128):
                        v_ps = psAv.tile([128, 128], BF, tag="v_ps")
                        nc.tensor.transpose(v_ps, vt_sb[:, k * 128:(k + 1) * 128], identity)
                        nc.vector.tensor_copy(out=V[:, Tt * 4 + k, :], in_=v_ps)

            # ---- phase B/C: attention + output projection ------------------
            with (
                tc.tile_pool(name="psst", bufs=2, space="PSUM") as psst,
                tc.tile_pool(name="psat", bufs=1, space="PSUM") as psat,
                tc.tile_pool(name="psz", bufs=1, space="PSUM") as psz,
                tc.tile_pool(name="psy", bufs=2, space="PSUM") as psy,
            ):
                for Tt in range(NT):
                    tsl = slice(Tt * TS, (Tt + 1) * TS)
                    att_sb = []
                    for h in range(HPG):
                        njj = 4 * Tt + 4          # s-tiles 0 .. 4*Tt+3
                        ngr = njj // 2
                        at_ps = psat.tile([128, TS], F32, tag="at")
                        z_ps = psz.tile([1, TS], F32, tag="z")
                        for g in range(ngr):
                            j0 = 2 * g
                            st = psst.tile([128, 1024], F32, tag="st")
                            for half in range(2):
                                j = j0 + half
                                nc.tensor.matmul(
                                    st[:, half * 512:(half + 1) * 512],
                                    KT[:, j * 128:(j + 1) * 128],
                                    QT[h][:, tsl],
                                    start=True, stop=True,
                                )
                            ex = expp.tile([128, 1024], BF, tag="ex")
                            nc.scalar.activation(
                                out=ex, in_=st,
                                func=mybir.ActivationFunctionType.Exp,
                                scale=SCALE,
                            )
                            if g == ngr - 2:
                                nc.vector.tensor_mul(ex, ex, maskA)
                            elif g == ngr - 1:
                                nc.vector.tensor_mul(ex, ex, maskB)
                            for half in range(2):
                                j = j0 + half
                                exh = ex[:, half * 512:(half + 1) * 512]
                                nc.tensor.matmul(
                                    z_ps, ones_s, exh,
                                    start=(j == 0), stop=(j == njj - 1),
                                )
                                nc.tensor.matmul(
                                    at_ps, V[:, j, :], exh,
                                    start=(j == 0), stop=(j == njj - 1),
                                )
                        zr = small.tile([1, TS], F32, tag="zr")
                        nc.vector.reciprocal(out=zr, in_=z_ps)
                        zrb = small.tile([1, TS], BF, tag="zrb")
                        nc.vector.tensor_copy(out=zrb, in_=zr)
                        zb_ps = psz.tile([128, TS], F32, tag="z")
                        nc.tensor.matmul(zb_ps, ones_r, zrb,
                                         start=True, stop=True)
                        zb_sb = small.tile([128, TS], BF, tag="zb_sb")
                        nc.vector.tensor_copy(out=zb_sb, in_=zb_ps)
                        at_sb = attp.tile([128, TS], BF, tag="at_sb")
                        nc.vector.tensor_mul(at_sb, at_ps, zb_sb)
                        att_sb.append(at_sb)
                    # output projection for these 512 rows
                    for fs in range(4):
                        fsl = slice(fs * 512, (fs + 1) * 512)
                        for tt in range(4):
                            y_ps = psy.tile([128, 512], F32, tag="y")
                            for h in range(HPG):
                                nc.tensor.matmul(
                                    y_ps,
                                    att_sb[h][:, tt * 128:(tt + 1) * 128],
                                    Wo_sb[:, h, fsl],
                                    start=(h == 0), stop=(h == HPG - 1),
                                )
                            y_sb = ypool.tile([128, 512], BF, tag="y_sb")
                            nc.vector.tensor_copy(out=y_sb, in_=y_ps)
                            nc.sync.dma_start(
                                out=yp[Tt * TS + tt * 128: Tt * TS + (tt + 1) * 128, fsl],
                                in_=y_sb,
                            )

            # ---- reduce partials across the 4 kv groups, return our share --
            nc.gpsimd.collective_compute(
                "ReduceScatter", mybir.AluOpType.add, replica_groups=GRP_BATCH,
                ins=[yp.opt()], outs=[yrs.opt()],
            )
            nc.gpsimd.dma_start(y, yrs[:])
    from concourse.bacc import _bass_rust
    _bass_rust.move_matmul_waits_to_ldweights(nc.m)
    _bass_rust.generate_event_semaphores(nc)
    _NC_CACHE["nc"] = nc
    return nc


def make_in_maps(x, Wq, bq, Wk, bk, Wv, bv, Wo, bo):
    bf = ml_dtypes.bfloat16
    x = np.asarray(x)
    xT = [np.ascontiguousarray(x[b].T, dtype=bf) for b in range(B)]
    Wq_bf = np.asarray(Wq).astype(bf)
    Wk_bf = np.asarray(Wk).astype(bf)
    Wv_bf = np.asarray(Wv).astype(bf)
    Wo_bf = np.asarray(Wo).astype(bf)
    bq = np.asarray(bq, dtype=np.float32)
    bk = np.asarray(bk, dtype=np.float32)
    bv = np.asarray(bv, dtype=np.float32)
    in_maps = []
    for c in range(8):
        b, g = divmod(c, NUM_KV)
        rq = slice(b * (D // 2), (b + 1) * (D // 2))
        in_maps.append({
            "xs": xT[b][g * TS:(g + 1) * TS],
            "Wqh": Wq_bf[rq, g * EG:(g + 1) * EG],
            "Wkvh": np.concatenate(
                [Wk_bf[rq, g * DH:(g + 1) * DH],
                 Wv_bf[rq, g * DH:(g + 1) * DH]], axis=1),
            "Woh": Wo_bf[g * EG + b * (EG // 2): g * EG + (b + 1) * (EG // 2), :],
            "bpack": np.concatenate(
                [bq[g * EG:(g + 1) * EG].reshape(4, DH).T,
                 bk[g * DH:(g + 1) * DH].reshape(DH, 1),
                 bv[g * DH:(g + 1) * DH].reshape(DH, 1)], axis=1
            ).astype(np.float32),
        })
    return in_maps


def gather(results, bo):
    bo = np.asarray(bo, dtype=np.float32)
    out = np.empty((B, T, D), dtype=np.float32)
    for b in range(B):
        for g in range(NUM_KV):
            out[b, g * TS:(g + 1) * TS] = results[b * NUM_KV + g]["y"]
        out[b] += bo[None, :]
    return out


def kernel(x, Wq, bq, Wk, bk, Wv, bv, Wo, bo):
    nc = build_nc()
    in_maps = make_in_maps(x, Wq, bq, Wk, bk, Wv, bv, Wo, bo)
    last = None
    for attempt in range(3):
        try:
            res = run_bass_kernel_spmd(nc, in_maps, list(range(8)))
            return gather(res.results, bo)
        except Exception as e:  # transient NRT_EXEC_UNIT_UNRECOVERABLE
            last = e
            import time as _t
            _t.sleep(10)
    raise last
